# revision 1
# baseline (speedup 1.0000x reference)
"""Trainium2 Bass kernel for nn_MatSurfGcn (GCN message passing, memory-bound).

Strategy (column-parallel over W_g1's output dim, 8 cores):
  reference =  enc -> gcn_conv(W_g1) -> gcn_conv(W_g2) -> head
  Both convs are linear and A @ (X @ W) == (A @ X) @ W, so the graph
  aggregation commutes out of the device entirely:
    x0  = relu(encoders)              [14, 4096]  (on-device, fp32, N=512 MMs)
    z_c = x0 @ W_g1_c                 [14, 1024]  (per-core column shard)
    u_c = z_c @ w2_c                  [14, 1]     (DVE mul+reduce)
    host: y = W_head.(A(A Su + b1.W_g2) + b_g2) + b_head   (two 14x14 matvecs)

  The big matmul streams W_g1 as a bf16 hi/lo pair (same 4 B/elem of HBM
  traffic as fp32 — the memory roofline is unchanged) with the activations
  packed [x_hi | pad | x_lo] into the PE's idle stationary columns, so the
  four cross terms (x_hi+x_lo)(W_hi+W_lo) all accumulate in one PSUM pass
  pair. bf16 passes run 1 cycle/row vs fp32's 4; end-to-end precision is
  ~1e-6 relative (errors cancel through the contraction).
"""

import os

import numpy as np

D1, D2 = 4096, 8192
N = 14
NCORES = 8
SH = D2 // NCORES        # 1024 W_g1 columns per core
KC = D1 // 128           # 32 contraction chunks of 128
CPT = 2                  # k-chunks per DMA tile (1 MiB bf16 hi/lo pairs)
WBUFS = int(os.environ.get("KERNEL_WBUFS", "6"))
ENC_K = 18               # 6+1 mats, 3+1 cyls, 4+1 planes, 1+1 power rows
XP = 46                  # packed stationary cols: hi 0:14, pad, lo 32:46
NT = SH // 512

_CACHE = {}


def _build_nc():
    import concourse.bacc as bacc
    import concourse.bass as bass
    import concourse.mybir as mybir
    import concourse.tile as tile

    f32 = mybir.dt.float32
    bf16 = mybir.dt.bfloat16
    relu = mybir.ActivationFunctionType.Relu
    psum = bass.MemorySpace.PSUM
    alu = mybir.AluOpType

    nc = bacc.Bacc(
        "TRN2", target_bir_lowering=False, debug=False, enable_asserts=False
    )

    wenc_d = nc.dram_tensor("wenc", [ENC_K, D1], f32, kind="ExternalInput")
    s_d = nc.dram_tensor("s", [ENC_K, N], f32, kind="ExternalInput")
    eye_d = nc.dram_tensor("eye", [N, N], f32, kind="ExternalInput")
    # bf16 [hi | lo] pairs, host-swizzled: row kt*128+p, col block
    # a*2*SH + half*SH + n  (kt = k-pair, a = k within pair)
    whl_d = nc.dram_tensor(
        "whl", [(KC // CPT) * 128, CPT * 2 * SH], bf16, kind="ExternalInput"
    )
    w2b_d = nc.dram_tensor("w2b", [N, SH], f32, kind="ExternalInput")
    t_d = nc.dram_tensor("t", [N, 1], f32, kind="ExternalOutput")

    JG = 4  # chunks per encoder block (512 cols)

    with tile.TileContext(nc) as tc:
        with (
            tc.tile_pool(name="const", bufs=1) as cpool,
            tc.tile_pool(name="whlp", bufs=WBUFS) as wpool,
            tc.tile_pool(name="encps", bufs=2, space=psum) as eps,
            tc.tile_pool(name="xtps", bufs=1, space=psum) as xtps,
            tc.tile_pool(name="zps", bufs=1, space=psum) as zps,
            tc.tile_pool(name="work", bufs=2) as sbp,
        ):
            wenc_sb = cpool.tile([ENC_K, D1], f32)
            s_sb = cpool.tile([ENC_K, N], f32)
            eye_sb = cpool.tile([N, N], f32)
            w2b_sb = cpool.tile([N, SH], f32)

            x0_sb = cpool.tile([N, D1], f32)
            # x0.T in one psum bank: chunk k at cols 14k..14k+14
            xT_ps = xtps.tile([128, KC * N], f32)
            xhl = cpool.tile([128, KC * XP], bf16)
            xhl_v = xhl[:, :].rearrange("p (k i) -> p k i", i=XP)
            nc.vector.memset(xhl_v[:, :, N:32], 0.0)  # pad cols stay finite
            xhi32 = cpool.tile([128, KC * N], f32)
            xlo32 = cpool.tile([128, KC * N], f32)
            z_ps = zps.tile([XP, SH], f32)

            # 3-stage software pipeline over 512-col groups j:
            #   stage A (j):   wenc DMA + encoder MM + relu
            #   stage B (j-1): PE transposes + DVE bf16 hi/lo pack
            #   stage C (j-2): 16 bf16 matmuls vs the streamed W tiles
            # Cross-engine handoffs (relu->transpose, pack->matmul) hide
            # behind the previous group's matmuls.
            NJ = D1 // 512
            wt_tiles = {}

            def stage_a(j):
                nc.sync.dma_start(
                    out=wenc_sb[:, j * 512 : (j + 1) * 512],
                    in_=wenc_d[:, j * 512 : (j + 1) * 512],
                )
                if j == 0:
                    nc.sync.dma_start(out=s_sb[:], in_=s_d[:])
                    nc.sync.dma_start(out=eye_sb[:], in_=eye_d[:])
                # prefetch this group's W tiles (consumed at stage C)
                for kt in (2 * j, 2 * j + 1):
                    wt = wpool.tile([128, CPT * 2 * SH], bf16, tag="wt")
                    nc.sync.dma_start(
                        out=wt[:], in_=whl_d[kt * 128 : (kt + 1) * 128, :]
                    )
                    wt_tiles[kt] = wt
                pe = eps.tile([N, 512], f32)
                nc.tensor.matmul(
                    pe[:],
                    s_sb[:],
                    wenc_sb[:, j * 512 : (j + 1) * 512],
                    start=True,
                    stop=True,
                )
                nc.scalar.activation(x0_sb[:, j * 512 : (j + 1) * 512], pe[:], relu)

            def stage_b(j):
                for kk in range(JG):
                    k = JG * j + kk
                    nc.tensor.transpose(
                        xT_ps[:, k * N : (k + 1) * N],
                        x0_sb[:, k * 128 : (k + 1) * 128],
                        eye_sb[:],
                    )
                gsl = slice(j * JG * N, (j + 1) * JG * N)
                src = xT_ps[:, gsl].rearrange("p (k i) -> p k i", i=N)
                hi_v = xhl_v[:, j * JG : (j + 1) * JG, 0:N]
                lo_v = xhl_v[:, j * JG : (j + 1) * JG, 32 : 32 + N]
                hi32_v = xhi32[:, gsl].rearrange("p (k i) -> p k i", i=N)
                lo32_v = xlo32[:, gsl].rearrange("p (k i) -> p k i", i=N)
                nc.vector.tensor_copy(hi_v, src)  # psum -> bf16
                nc.vector.tensor_copy(hi32_v, hi_v)  # back to f32
                nc.vector.tensor_sub(lo32_v, src, hi32_v)
                nc.vector.tensor_copy(lo_v, lo32_v)  # -> bf16

            def stage_c(j):
                for kt in (2 * j, 2 * j + 1):
                    wt = wt_tiles.pop(kt)
                    for a in range(CPT):
                        k = kt * CPT + a
                        for half in range(2):
                            for nt in range(NT):
                                off = a * 2 * SH + half * SH + nt * 512
                                nc.tensor.matmul(
                                    z_ps[:, nt * 512 : (nt + 1) * 512],
                                    xhl[:, k * XP : (k + 1) * XP],
                                    wt[:, off : off + 512],
                                    start=(k == 0 and half == 0),
                                    stop=(k == KC - 1 and half == 1),
                                )

            for j in range(NJ + 2):
                if j < NJ:
                    stage_a(j)
                if 1 <= j <= NJ:
                    stage_b(j - 1)
                if j >= 2:
                    stage_c(j - 2)

            nc.sync.dma_start(out=w2b_sb[:], in_=w2b_d[:])

            # ---- z = hi rows + lo rows, then contract with w2 ----
            zlo = sbp.tile([N, SH], f32, tag="zlo")
            zz = sbp.tile([N, SH], f32, tag="zz")
            for nt in range(NT):
                sl = slice(nt * 512, (nt + 1) * 512)
                nc.scalar.copy(zlo[:, sl], z_ps[32 : 32 + N, sl])
                nc.vector.tensor_add(zz[:, sl], z_ps[0:N, sl], zlo[:, sl])
            prod = sbp.tile([N, SH], f32, tag="prod")
            nc.vector.tensor_mul(prod[:], zz[:], w2b_sb[:])
            t_sb = sbp.tile([N, 1], f32, tag="tsb")
            nc.vector.tensor_reduce(
                t_sb[:], prod[:], axis=mybir.AxisListType.X, op=alu.add
            )
            nc.sync.dma_start(out=t_d[:], in_=t_sb[:])

    nc.compile()
    return nc


def get_nc():
    if "nc" not in _CACHE:
        _CACHE["nc"] = _build_nc()
    return _CACHE["nc"]


def build_graph_matrix(edge_index):
    """Dense normalized adjacency of the PyG-style GCNConv (self-loops +
    symmetric deg^{-1/2}); multi-edges accumulate like segment_sum does."""
    ei = np.concatenate(
        [edge_index.astype(np.int64), np.stack([np.arange(N), np.arange(N)])],
        axis=1,
    )
    src, dst = ei[0], ei[1]
    deg = np.zeros(N, np.float32)
    np.add.at(deg, dst, np.ones(len(dst), np.float32))
    dis = np.where(deg > 0, 1.0 / np.sqrt(np.maximum(deg, 1e-12)), 0.0).astype(
        np.float32
    )
    A = np.zeros((N, N), np.float32)
    np.add.at(A, (dst, src), dis[src] * dis[dst])
    return A


def build_host_inputs(inputs):
    """Per-core input maps + the graph matrix for the host epilogue."""
    f32 = np.float32
    import ml_dtypes

    bf16 = ml_dtypes.bfloat16
    mats = np.asarray(inputs["mats"], f32)
    cyls = np.asarray(inputs["cyls"], f32)
    planes = np.asarray(inputs["planes"], f32)
    power = np.asarray(inputs["power"], f32)
    edge_index = np.asarray(inputs["edge_index"])

    A = build_graph_matrix(edge_index)

    # Block-diagonal node features with bias rows of ones: x0 = relu(S.T @ Wenc)
    S = np.zeros((ENC_K, N), f32)
    S[0:6, 0:6] = mats.T
    S[6, 0:6] = 1.0
    S[7:10, 6:10] = cyls.T
    S[10, 6:10] = 1.0
    S[11:15, 10:13] = planes.T
    S[15, 10:13] = 1.0
    S[16, 13] = power[0] / 10000.0
    S[17, 13] = 1.0

    Wenc = np.ascontiguousarray(
        np.concatenate(
            [
                np.asarray(inputs["W_mat"], f32),
                np.asarray(inputs["b_mat"], f32)[None, :],
                np.asarray(inputs["W_cyl"], f32),
                np.asarray(inputs["b_cyl"], f32)[None, :],
                np.asarray(inputs["W_pl"], f32),
                np.asarray(inputs["b_pl"], f32)[None, :],
                np.asarray(inputs["W_pw"], f32),
                np.asarray(inputs["b_pw"], f32)[None, :],
            ],
            axis=0,
        )
    )
    assert Wenc.shape == (ENC_K, D1)

    W_g1 = np.asarray(inputs["W_g1"], f32)
    W_g2 = np.asarray(inputs["W_g2"], f32)

    in_maps = []
    for c in range(NCORES):
        sl = slice(c * SH, (c + 1) * SH)
        Wc = W_g1[:, sl]
        Whi = Wc.astype(bf16)
        Wlo = (Wc - Whi.astype(f32)).astype(bf16)
        # per chunk k: [hi(1024) | lo(1024)]; swizzle pairs of chunks
        whl = np.concatenate(
            [Whi.reshape(KC, 128, SH), Wlo.reshape(KC, 128, SH)], axis=2
        )  # [KC, 128, 2*SH]
        whl = np.ascontiguousarray(
            whl.reshape(KC // CPT, CPT, 128, 2 * SH)
            .transpose(0, 2, 1, 3)
            .reshape((KC // CPT) * 128, CPT * 2 * SH)
        )
        w2b_c = np.ascontiguousarray(np.tile(W_g2[sl, 0][None, :], (N, 1)))
        in_maps.append(
            {
                "wenc": Wenc,
                "s": S,
                "eye": np.eye(N, dtype=f32),
                "whl": whl,
                "w2b": w2b_c,
            }
        )
    return in_maps, A


def epilogue(t_parts, A, inputs):
    f32 = np.float32
    b_g1 = np.asarray(inputs["b_g1"], f32)
    W_g2 = np.asarray(inputs["W_g2"], f32)
    b_g2 = np.asarray(inputs["b_g2"], f32)
    W_head = np.asarray(inputs["W_head"], f32)
    b_head = np.asarray(inputs["b_head"], f32)
    u = np.add.reduce([p.astype(f32) for p in t_parts])  # [14,1] un-aggregated
    t_full = A @ u + np.float32(b_g1 @ W_g2[:, 0])  # conv2 input = x1 @ W_g2
    x2 = A @ t_full + b_g2[0]
    y = float(x2[:, 0] @ W_head[:, 0]) + float(b_head[0])
    return np.array([y], dtype=f32)


def run_on_hw(in_maps, trace=False, tmpdir=None):
    from concourse.bass_utils import run_bass_kernel_spmd

    nc = get_nc()
    return run_bass_kernel_spmd(
        nc,
        in_maps,
        core_ids=list(range(NCORES)),
        trace=trace,
        tmpdir=tmpdir,
    )


def kernel(**inputs):
    in_maps, A = build_host_inputs(inputs)
    res = run_on_hw(in_maps, trace=bool(int(os.environ.get("KERNEL_TRACE", "0"))))
    _CACHE["last_result"] = res
    t_parts = [r["t"] for r in res.results]
    return epilogue(t_parts, A, inputs)



# revision 2
# speedup vs baseline: 1.1911x; 1.1911x over previous
"""Trainium2 Bass kernel for nn_MatSurfGcn (GCN message passing, memory-bound).

Strategy (column-parallel over W_g1's output dim, 8 cores):
  reference =  enc -> gcn_conv(W_g1) -> gcn_conv(W_g2) -> head
  Both convs are linear and A @ (X @ W) == (A @ X) @ W, so the graph
  aggregation commutes out of the device entirely.  Per core c:
    x0T = relu(Wenc.T @ S)            [4096, 14]   (32 tiny PE matmuls,
                                                    written transposed)
    z_c = x0T.T @ Wv_c                [14, 1024]   Wv_c = W_g1[:,c] * w2_c
    t_c = row_sum(z_c)                [14, 1]
    host: y = head(A(A Su + b1.W_g2) + b_g2)       (two 14x14 matvecs)

  W_g2 is folded into W_g1's columns on the host (same device FLOPs, kills
  the tail multiply), and the result streams as plain bf16: 8.4 MB/core,
  so DMA (~23.4 us at 358 GB/s) and the PE bf16 column stream (~14 us)
  are both near the memory roofline.  End-to-end error ~3e-3 vs the 2e-2
  gate (bf16 quantization of x and W does not average down through the
  random-sign contraction, but starts 6x under the gate).
"""

import os

import numpy as np

D1, D2 = 4096, 8192
N = 14
NCORES = 8
SH = D2 // NCORES        # 1024 W_g1 columns per core
KC = D1 // 128           # 32 contraction chunks of 128
CPT = 2                  # k-chunks per DMA tile (512 KiB bf16)
NTILES = KC // CPT       # 16 streamed W tiles
WBUFS = int(os.environ.get("KERNEL_WBUFS", "4"))
ENC_K = 18               # 6+1 mats, 3+1 cyls, 4+1 planes, 1+1 power rows

_CACHE = {}


def _build_nc():
    import concourse.bacc as bacc
    import concourse.bass as bass
    import concourse.mybir as mybir
    import concourse.tile as tile

    f32 = mybir.dt.float32
    bf16 = mybir.dt.bfloat16
    relu = mybir.ActivationFunctionType.Relu
    psum = bass.MemorySpace.PSUM
    alu = mybir.AluOpType

    nc = bacc.Bacc(
        "TRN2", target_bir_lowering=False, debug=False, enable_asserts=False
    )

    wenc_d = nc.dram_tensor("wenc", [ENC_K, D1], f32, kind="ExternalInput")
    s_d = nc.dram_tensor("s", [ENC_K, N], f32, kind="ExternalInput")
    # host-swizzled bf16 W_g1 shard with w2 folded in:
    # wv[p, k*SH + j] = (W_g1[k*128+p, c*SH+j] * w2[c*SH+j]) as bf16
    wv_d = nc.dram_tensor("wv", [128, KC * SH], bf16, kind="ExternalInput")
    t_d = nc.dram_tensor("t", [N, 1], f32, kind="ExternalOutput")

    with tile.TileContext(nc) as tc:
        with (
            tc.tile_pool(name="const", bufs=1) as cpool,
            tc.tile_pool(name="wvp", bufs=WBUFS) as wpool,
            tc.tile_pool(name="xps", bufs=1, space=psum) as xps,
            tc.tile_pool(name="zps", bufs=1, space=psum) as zps,
        ):
            wenc_sb = cpool.tile([ENC_K, D1], f32)
            s_sb = cpool.tile([ENC_K, N], f32)
            xb = cpool.tile([128, KC * N], bf16)
            t_sb = cpool.tile([N, 1], f32)
            xT_ps = xps.tile([128, KC * N], f32)
            z_ps = zps.tile([N, SH], f32)

            # W stream first: 16 x 512 KiB on the sync HWDGE ring keeps the
            # DMA engines saturated for the whole kernel.
            wts = []
            for g in range(NTILES):
                wt = wpool.tile([128, CPT * SH], bf16, tag="wt")
                nc.sync.dma_start(
                    out=wt[:], in_=wv_d[:, g * CPT * SH : (g + 1) * CPT * SH]
                )
                wts.append(wt)
            # small encoder inputs ride the scalar-engine HWDGE ring
            nc.scalar.dma_start(out=wenc_sb[:], in_=wenc_d[:])
            nc.scalar.dma_start(out=s_sb[:], in_=s_d[:])

            # x0T = relu(Wenc.T @ S), produced already transposed so the big
            # matmul needs no PE transposes: chunk k lands at cols 14k..14k+14
            for k in range(KC):
                nc.tensor.matmul(
                    xT_ps[:, k * N : (k + 1) * N],
                    wenc_sb[:, k * 128 : (k + 1) * 128],
                    s_sb[:],
                    start=True,
                    stop=True,
                )
            nc.scalar.activation(xb[:], xT_ps[:], relu)

            # z += x0T_k.T @ Wv_k, accumulated across all 32 k-chunks in PSUM
            for g in range(NTILES):
                wt = wts[g]
                for a in range(CPT):
                    k = g * CPT + a
                    for nt in range(2):
                        nc.tensor.matmul(
                            z_ps[:, nt * 512 : (nt + 1) * 512],
                            xb[:, k * N : (k + 1) * N],
                            wt[:, a * SH + nt * 512 : a * SH + (nt + 1) * 512],
                            start=(k == 0),
                            stop=(k == KC - 1),
                        )

            # t = row_sum(z)  (w2 already folded into the W stream)
            nc.vector.tensor_reduce(
                t_sb[:], z_ps[:], axis=mybir.AxisListType.X, op=alu.add
            )
            nc.sync.dma_start(out=t_d[:], in_=t_sb[:])

    nc.compile()
    return nc


def get_nc():
    if "nc" not in _CACHE:
        _CACHE["nc"] = _build_nc()
    return _CACHE["nc"]


def build_graph_matrix(edge_index):
    """Dense normalized adjacency of the PyG-style GCNConv (self-loops +
    symmetric deg^{-1/2}); multi-edges accumulate like segment_sum does."""
    ei = np.concatenate(
        [edge_index.astype(np.int64), np.stack([np.arange(N), np.arange(N)])],
        axis=1,
    )
    src, dst = ei[0], ei[1]
    deg = np.zeros(N, np.float32)
    np.add.at(deg, dst, np.ones(len(dst), np.float32))
    dis = np.where(deg > 0, 1.0 / np.sqrt(np.maximum(deg, 1e-12)), 0.0).astype(
        np.float32
    )
    A = np.zeros((N, N), np.float32)
    np.add.at(A, (dst, src), dis[src] * dis[dst])
    return A


def build_host_inputs(inputs):
    """Per-core input maps + the graph matrix for the host epilogue."""
    f32 = np.float32
    import ml_dtypes

    bf16 = ml_dtypes.bfloat16
    mats = np.asarray(inputs["mats"], f32)
    cyls = np.asarray(inputs["cyls"], f32)
    planes = np.asarray(inputs["planes"], f32)
    power = np.asarray(inputs["power"], f32)
    edge_index = np.asarray(inputs["edge_index"])

    A = build_graph_matrix(edge_index)

    # Block-diagonal node features with bias rows of ones: x0 = relu(S.T @ Wenc)
    S = np.zeros((ENC_K, N), f32)
    S[0:6, 0:6] = mats.T
    S[6, 0:6] = 1.0
    S[7:10, 6:10] = cyls.T
    S[10, 6:10] = 1.0
    S[11:15, 10:13] = planes.T
    S[15, 10:13] = 1.0
    S[16, 13] = power[0] / 10000.0
    S[17, 13] = 1.0

    Wenc = np.ascontiguousarray(
        np.concatenate(
            [
                np.asarray(inputs["W_mat"], f32),
                np.asarray(inputs["b_mat"], f32)[None, :],
                np.asarray(inputs["W_cyl"], f32),
                np.asarray(inputs["b_cyl"], f32)[None, :],
                np.asarray(inputs["W_pl"], f32),
                np.asarray(inputs["b_pl"], f32)[None, :],
                np.asarray(inputs["W_pw"], f32),
                np.asarray(inputs["b_pw"], f32)[None, :],
            ],
            axis=0,
        )
    )
    assert Wenc.shape == (ENC_K, D1)

    W_g1 = np.asarray(inputs["W_g1"], f32)
    W_g2 = np.asarray(inputs["W_g2"], f32)

    in_maps = []
    for c in range(NCORES):
        sl = slice(c * SH, (c + 1) * SH)
        wv = (W_g1[:, sl] * W_g2[sl, 0][None, :]).astype(bf16)  # [D1, SH]
        # swizzle so chunk k sits at cols k*SH..(k+1)*SH of a [128, *] image
        wsw = np.ascontiguousarray(
            wv.reshape(KC, 128, SH).transpose(1, 0, 2).reshape(128, KC * SH)
        )
        in_maps.append({"wenc": Wenc, "s": S, "wv": wsw})
    return in_maps, A


def epilogue(t_parts, A, inputs):
    f32 = np.float32
    b_g1 = np.asarray(inputs["b_g1"], f32)
    W_g2 = np.asarray(inputs["W_g2"], f32)
    b_g2 = np.asarray(inputs["b_g2"], f32)
    W_head = np.asarray(inputs["W_head"], f32)
    b_head = np.asarray(inputs["b_head"], f32)
    u = np.add.reduce([p.astype(f32) for p in t_parts])  # [14,1] un-aggregated
    t_full = A @ u + np.float32(b_g1 @ W_g2[:, 0])  # conv2 input = x1 @ W_g2
    x2 = A @ t_full + b_g2[0]
    y = float(x2[:, 0] @ W_head[:, 0]) + float(b_head[0])
    return np.array([y], dtype=f32)


def run_on_hw(in_maps, trace=False, tmpdir=None):
    from concourse.bass_utils import run_bass_kernel_spmd

    nc = get_nc()
    return run_bass_kernel_spmd(
        nc,
        in_maps,
        core_ids=list(range(NCORES)),
        trace=trace,
        tmpdir=tmpdir,
    )


def kernel(**inputs):
    in_maps, A = build_host_inputs(inputs)
    res = run_on_hw(in_maps, trace=bool(int(os.environ.get("KERNEL_TRACE", "0"))))
    _CACHE["last_result"] = res
    t_parts = [r["t"] for r in res.results]
    return epilogue(t_parts, A, inputs)


# revision 5
# speedup vs baseline: 1.8448x; 1.5487x over previous
"""Trainium2 Bass kernel for nn_MatSurfGcn (GCN message passing, memory-bound).

Strategy (column-parallel over W_g1's output dim, 8 cores):
  reference =  enc -> gcn_conv(W_g1) -> gcn_conv(W_g2) -> head
  Both convs are linear and A @ (X @ W) == (A @ X) @ W, so the graph
  aggregation commutes out of the device entirely.  Per core c:
    x0T = relu(Wenc.T @ S)            [4096, 14]   (32 tiny PE matmuls,
                                                    written transposed)
    z_c = x0T.T @ Wv_c                [14, 1024]   Wv_c = W_g1[:,c] * w2_c
    t_c = row_sum(z_c)                [14, 1]
    host: y = head(A(A Su + b1.W_g2) + b_g2)       (two 14x14 matvecs)

  W_g2 is folded into W_g1's columns on the host (same device FLOPs, kills
  the tail multiply), and the result streams as plain bf16: 8.4 MB/core,
  so DMA (~23.4 us at 358 GB/s) and the PE bf16 column stream (~14 us)
  are both near the memory roofline.  End-to-end error ~3e-3 vs the 2e-2
  gate (bf16 quantization of x and W does not average down through the
  random-sign contraction, but starts 6x under the gate).

  The 14-node activations x0T are computed on the host and replicated to
  all cores (per the sharding hint): on the PE the encoder decomposes
  into 32 latency-bound 14-column matmuls (~13 us) that would gate the
  first z matmul and starve the W stream, while as data it is a 114 KiB
  bf16 DMA that rides along with the first W tile.
"""

import os

import numpy as np

D1, D2 = 4096, 8192
N = 14
NCORES = 8
SH = D2 // NCORES        # 1024 W_g1 columns per core
KC = D1 // 128           # 32 contraction chunks of 128
CPT = 2                  # k-chunks per DMA tile (512 KiB bf16)
NTILES = KC // CPT       # 16 streamed W tiles
WBUFS = int(os.environ.get("KERNEL_WBUFS", "4"))
ENC_K = 18               # 6+1 mats, 3+1 cyls, 4+1 planes, 1+1 power rows

_CACHE = {}


def _build_nc():
    import concourse.bacc as bacc
    import concourse.bass as bass
    import concourse.mybir as mybir
    import concourse.tile as tile

    f32 = mybir.dt.float32
    bf16 = mybir.dt.bfloat16
    psum = bass.MemorySpace.PSUM
    alu = mybir.AluOpType

    nc = bacc.Bacc(
        "TRN2", target_bir_lowering=False, debug=False, enable_asserts=False
    )

    # host-computed x0T = relu(enc(inputs)).T, swizzled so chunk k sits at
    # cols 14k..14k+14: xb[p, k*N + n] = x0T[k*128 + p, n]
    xb_d = nc.dram_tensor("xb", [128, KC * N], bf16, kind="ExternalInput")
    # host-swizzled bf16 W_g1 shard with w2 folded in:
    # wv[p, k*SH + j] = (W_g1[k*128+p, c*SH+j] * w2[c*SH+j]) as bf16
    wv_d = nc.dram_tensor("wv", [128, KC * SH], bf16, kind="ExternalInput")
    t_d = nc.dram_tensor("t", [N, 1], f32, kind="ExternalOutput")

    with tile.TileContext(nc) as tc:
        with (
            tc.tile_pool(name="const", bufs=1) as cpool,
            tc.tile_pool(name="wvp", bufs=WBUFS) as wpool,
            tc.tile_pool(name="zps", bufs=1, space=psum) as zps,
        ):
            xb = cpool.tile([128, KC * N], bf16)
            t_sb = cpool.tile([N, 1], f32)
            z_ps = zps.tile([N, SH], f32)

            # W stream first: 16 x 512 KiB on the sync HWDGE ring keeps the
            # DMA engines saturated for the whole kernel; the small xb rides
            # the scalar-engine ring in parallel.
            wts = []
            for g in range(NTILES):
                wt = wpool.tile([128, CPT * SH], bf16, tag="wt")
                nc.sync.dma_start(
                    out=wt[:], in_=wv_d[:, g * CPT * SH : (g + 1) * CPT * SH]
                )
                wts.append(wt)
            nc.scalar.dma_start(out=xb[:], in_=xb_d[:])

            # z += x0T_k.T @ Wv_k, accumulated across all 32 k-chunks in PSUM
            for g in range(NTILES):
                wt = wts[g]
                for a in range(CPT):
                    k = g * CPT + a
                    for nt in range(2):
                        nc.tensor.matmul(
                            z_ps[:, nt * 512 : (nt + 1) * 512],
                            xb[:, k * N : (k + 1) * N],
                            wt[:, a * SH + nt * 512 : a * SH + (nt + 1) * 512],
                            start=(k == 0),
                            stop=(k == KC - 1),
                        )

            # t = row_sum(z)  (w2 already folded into the W stream)
            nc.vector.tensor_reduce(
                t_sb[:], z_ps[:], axis=mybir.AxisListType.X, op=alu.add
            )
            nc.sync.dma_start(out=t_d[:], in_=t_sb[:])

    nc.compile()
    return nc


def get_nc():
    if "nc" not in _CACHE:
        _CACHE["nc"] = _build_nc()
    return _CACHE["nc"]


def build_graph_matrix(edge_index):
    """Dense normalized adjacency of the PyG-style GCNConv (self-loops +
    symmetric deg^{-1/2}); multi-edges accumulate like segment_sum does."""
    ei = np.concatenate(
        [edge_index.astype(np.int64), np.stack([np.arange(N), np.arange(N)])],
        axis=1,
    )
    src, dst = ei[0], ei[1]
    deg = np.zeros(N, np.float32)
    np.add.at(deg, dst, np.ones(len(dst), np.float32))
    dis = np.where(deg > 0, 1.0 / np.sqrt(np.maximum(deg, 1e-12)), 0.0).astype(
        np.float32
    )
    A = np.zeros((N, N), np.float32)
    np.add.at(A, (dst, src), dis[src] * dis[dst])
    return A


def build_host_inputs(inputs):
    """Per-core input maps + the graph matrix for the host epilogue."""
    f32 = np.float32
    import ml_dtypes

    bf16 = ml_dtypes.bfloat16
    mats = np.asarray(inputs["mats"], f32)
    cyls = np.asarray(inputs["cyls"], f32)
    planes = np.asarray(inputs["planes"], f32)
    power = np.asarray(inputs["power"], f32)
    edge_index = np.asarray(inputs["edge_index"])

    A = build_graph_matrix(edge_index)

    # Block-diagonal node features with bias rows of ones: x0 = relu(S.T @ Wenc)
    S = np.zeros((ENC_K, N), f32)
    S[0:6, 0:6] = mats.T
    S[6, 0:6] = 1.0
    S[7:10, 6:10] = cyls.T
    S[10, 6:10] = 1.0
    S[11:15, 10:13] = planes.T
    S[15, 10:13] = 1.0
    S[16, 13] = power[0] / 10000.0
    S[17, 13] = 1.0

    Wenc = np.ascontiguousarray(
        np.concatenate(
            [
                np.asarray(inputs["W_mat"], f32),
                np.asarray(inputs["b_mat"], f32)[None, :],
                np.asarray(inputs["W_cyl"], f32),
                np.asarray(inputs["b_cyl"], f32)[None, :],
                np.asarray(inputs["W_pl"], f32),
                np.asarray(inputs["b_pl"], f32)[None, :],
                np.asarray(inputs["W_pw"], f32),
                np.asarray(inputs["b_pw"], f32)[None, :],
            ],
            axis=0,
        )
    )
    assert Wenc.shape == (ENC_K, D1)

    W_g1 = np.asarray(inputs["W_g1"], f32)
    W_g2 = np.asarray(inputs["W_g2"], f32)

    # replicated 14-node activations, transposed + chunk-swizzled for the PE
    x0 = np.maximum(S.T @ Wenc, 0.0)  # [N, D1]
    xb = np.ascontiguousarray(
        x0.T.reshape(KC, 128, N).transpose(1, 0, 2).reshape(128, KC * N)
    ).astype(bf16)

    in_maps = []
    for c in range(NCORES):
        sl = slice(c * SH, (c + 1) * SH)
        wv = (W_g1[:, sl] * W_g2[sl, 0][None, :]).astype(bf16)  # [D1, SH]
        # swizzle so chunk k sits at cols k*SH..(k+1)*SH of a [128, *] image
        wsw = np.ascontiguousarray(
            wv.reshape(KC, 128, SH).transpose(1, 0, 2).reshape(128, KC * SH)
        )
        in_maps.append({"xb": xb, "wv": wsw})
    return in_maps, A


def epilogue(t_parts, A, inputs):
    f32 = np.float32
    b_g1 = np.asarray(inputs["b_g1"], f32)
    W_g2 = np.asarray(inputs["W_g2"], f32)
    b_g2 = np.asarray(inputs["b_g2"], f32)
    W_head = np.asarray(inputs["W_head"], f32)
    b_head = np.asarray(inputs["b_head"], f32)
    u = np.add.reduce([p.astype(f32) for p in t_parts])  # [14,1] un-aggregated
    t_full = A @ u + np.float32(b_g1 @ W_g2[:, 0])  # conv2 input = x1 @ W_g2
    x2 = A @ t_full + b_g2[0]
    y = float(x2[:, 0] @ W_head[:, 0]) + float(b_head[0])
    return np.array([y], dtype=f32)


def run_on_hw(in_maps, trace=False, tmpdir=None):
    from concourse.bass_utils import run_bass_kernel_spmd

    nc = get_nc()
    return run_bass_kernel_spmd(
        nc,
        in_maps,
        core_ids=list(range(NCORES)),
        trace=trace,
        tmpdir=tmpdir,
    )


def kernel(**inputs):
    in_maps, A = build_host_inputs(inputs)
    res = run_on_hw(in_maps, trace=bool(int(os.environ.get("KERNEL_TRACE", "0"))))
    _CACHE["last_result"] = res
    t_parts = [r["t"] for r in res.results]
    return epilogue(t_parts, A, inputs)


# revision 7
# speedup vs baseline: 1.8877x; 1.0233x over previous
"""Trainium2 Bass kernel for nn_MatSurfGcn (GCN message passing, memory-bound).

Strategy (column-parallel over W_g1's output dim, 8 cores):
  reference =  enc -> gcn_conv(W_g1) -> gcn_conv(W_g2) -> head
  Both convs are linear and A @ (X @ W) == (A @ X) @ W, so the graph
  aggregation commutes out of the device entirely.  Per core c:
    x0T = relu(Wenc.T @ S)            [4096, 14]   (32 tiny PE matmuls,
                                                    written transposed)
    z_c = x0T.T @ Wv_c                [14, 1024]   Wv_c = W_g1[:,c] * w2_c
    t_c = row_sum(z_c)                [14, 1]
    host: y = head(A(A Su + b1.W_g2) + b_g2)       (two 14x14 matvecs)

  W_g2 is folded into W_g1's columns on the host (same device FLOPs, kills
  the tail multiply), and the result streams as plain bf16: 8.4 MB/core,
  so DMA (~23.4 us at 358 GB/s) and the PE bf16 column stream (~14 us)
  are both near the memory roofline.  End-to-end error ~3e-3 vs the 2e-2
  gate (bf16 quantization of x and W does not average down through the
  random-sign contraction, but starts 6x under the gate).

  The 14-node activations x0T are computed on the host and replicated to
  all cores (per the sharding hint): on the PE the encoder decomposes
  into 32 latency-bound 14-column matmuls (~13 us) that would gate the
  first z matmul and starve the W stream, while as data it is a 114 KiB
  bf16 DMA that rides along with the first W tile.
"""

import os

import numpy as np

D1, D2 = 4096, 8192
N = 14
NCORES = 8
SH = D2 // NCORES        # 1024 W_g1 columns per core
KC = D1 // 128           # 32 contraction chunks of 128
CPT = 2                  # k-chunks per DMA tile (512 KiB bf16)
NFULL = 15               # full 2-chunk tiles; last 2 chunks go as 1-chunk
WBUFS = int(os.environ.get("KERNEL_WBUFS", "6"))
ENC_K = 18               # 6+1 mats, 3+1 cyls, 4+1 planes, 1+1 power rows

_CACHE = {}


def _build_nc():
    import concourse.bacc as bacc
    import concourse.bass as bass
    import concourse.mybir as mybir
    import concourse.tile as tile

    f32 = mybir.dt.float32
    bf16 = mybir.dt.bfloat16
    psum = bass.MemorySpace.PSUM
    alu = mybir.AluOpType

    nc = bacc.Bacc(
        "TRN2", target_bir_lowering=False, debug=False, enable_asserts=False
    )

    # host-computed x0T = relu(enc(inputs)).T, swizzled so chunk k sits at
    # cols 14k..14k+14: xb[p, k*N + n] = x0T[k*128 + p, n]
    xb_d = nc.dram_tensor("xb", [128, KC * N], bf16, kind="ExternalInput")
    # host-swizzled bf16 W_g1 shard with w2 folded in:
    # wv[p, k*SH + j] = (W_g1[k*128+p, c*SH+j] * w2[c*SH+j]) as bf16
    wv_d = nc.dram_tensor("wv", [128, KC * SH], bf16, kind="ExternalInput")
    t_d = nc.dram_tensor("t", [N, 1], f32, kind="ExternalOutput")

    with tile.TileContext(nc) as tc:
        with (
            tc.tile_pool(name="const", bufs=1) as cpool,
            tc.tile_pool(name="wvp", bufs=WBUFS) as wpool,
            tc.tile_pool(name="zps", bufs=1, space=psum) as zps,
        ):
            xb = cpool.tile([128, KC * N], bf16)
            t_sb = cpool.tile([N, 1], f32)
            ta = cpool.tile([N, 1], f32)
            tb = cpool.tile([N, 1], f32)
            red_sc = cpool.tile([N, 512], f32)
            z_ps = zps.tile([N, SH], f32)

            # xb first (it gates the first matmul), then the W stream:
            # 15 x 512 KiB + 2 x 256 KiB on the sync HWDGE ring keeps the
            # DMA engines saturated for the whole kernel.
            nc.sync.dma_start(out=xb[:], in_=xb_d[:])
            tiles = []  # (wt, k0, nchunks)
            for g in range(NFULL):
                wt = wpool.tile([128, CPT * SH], bf16, tag="wt")
                nc.sync.dma_start(
                    out=wt[:], in_=wv_d[:, g * CPT * SH : (g + 1) * CPT * SH]
                )
                tiles.append((wt, g * CPT, CPT))
            for k0 in range(NFULL * CPT, KC):
                wt = wpool.tile([128, SH], bf16, tag="wl")
                nc.sync.dma_start(
                    out=wt[:], in_=wv_d[:, k0 * SH : (k0 + 1) * SH]
                )
                tiles.append((wt, k0, 1))

            # z += x0T_k.T @ Wv_k, accumulated across all 32 k-chunks in PSUM
            for wt, k0, nch in tiles:
                for a in range(nch):
                    k = k0 + a
                    for nt in range(2):
                        nc.tensor.matmul(
                            z_ps[:, nt * 512 : (nt + 1) * 512],
                            xb[:, k * N : (k + 1) * N],
                            wt[:, a * SH + nt * 512 : a * SH + (nt + 1) * 512],
                            start=(k == 0),
                            stop=(k == KC - 1),
                        )

            # t = row_sum(z) (w2 already folded into the W stream), halves
            # reduced on DVE and ACT in parallel to shorten the tail
            nc.vector.tensor_reduce(
                ta[:], z_ps[:, 0:512], axis=mybir.AxisListType.X, op=alu.add
            )
            nc.scalar.activation(
                red_sc[:],
                z_ps[:, 512:1024],
                mybir.ActivationFunctionType.Copy,
                accum_out=tb[:],
            )
            nc.vector.tensor_add(t_sb[:], ta[:], tb[:])
            nc.sync.dma_start(out=t_d[:], in_=t_sb[:])

    nc.compile()
    return nc


def get_nc():
    if "nc" not in _CACHE:
        _CACHE["nc"] = _build_nc()
    return _CACHE["nc"]


def build_graph_matrix(edge_index):
    """Dense normalized adjacency of the PyG-style GCNConv (self-loops +
    symmetric deg^{-1/2}); multi-edges accumulate like segment_sum does."""
    ei = np.concatenate(
        [edge_index.astype(np.int64), np.stack([np.arange(N), np.arange(N)])],
        axis=1,
    )
    src, dst = ei[0], ei[1]
    deg = np.zeros(N, np.float32)
    np.add.at(deg, dst, np.ones(len(dst), np.float32))
    dis = np.where(deg > 0, 1.0 / np.sqrt(np.maximum(deg, 1e-12)), 0.0).astype(
        np.float32
    )
    A = np.zeros((N, N), np.float32)
    np.add.at(A, (dst, src), dis[src] * dis[dst])
    return A


def build_host_inputs(inputs):
    """Per-core input maps + the graph matrix for the host epilogue."""
    f32 = np.float32
    import ml_dtypes

    bf16 = ml_dtypes.bfloat16
    mats = np.asarray(inputs["mats"], f32)
    cyls = np.asarray(inputs["cyls"], f32)
    planes = np.asarray(inputs["planes"], f32)
    power = np.asarray(inputs["power"], f32)
    edge_index = np.asarray(inputs["edge_index"])

    A = build_graph_matrix(edge_index)

    # Block-diagonal node features with bias rows of ones: x0 = relu(S.T @ Wenc)
    S = np.zeros((ENC_K, N), f32)
    S[0:6, 0:6] = mats.T
    S[6, 0:6] = 1.0
    S[7:10, 6:10] = cyls.T
    S[10, 6:10] = 1.0
    S[11:15, 10:13] = planes.T
    S[15, 10:13] = 1.0
    S[16, 13] = power[0] / 10000.0
    S[17, 13] = 1.0

    Wenc = np.ascontiguousarray(
        np.concatenate(
            [
                np.asarray(inputs["W_mat"], f32),
                np.asarray(inputs["b_mat"], f32)[None, :],
                np.asarray(inputs["W_cyl"], f32),
                np.asarray(inputs["b_cyl"], f32)[None, :],
                np.asarray(inputs["W_pl"], f32),
                np.asarray(inputs["b_pl"], f32)[None, :],
                np.asarray(inputs["W_pw"], f32),
                np.asarray(inputs["b_pw"], f32)[None, :],
            ],
            axis=0,
        )
    )
    assert Wenc.shape == (ENC_K, D1)

    W_g1 = np.asarray(inputs["W_g1"], f32)
    W_g2 = np.asarray(inputs["W_g2"], f32)

    # replicated 14-node activations, transposed + chunk-swizzled for the PE
    x0 = np.maximum(S.T @ Wenc, 0.0)  # [N, D1]
    xb = np.ascontiguousarray(
        x0.T.reshape(KC, 128, N).transpose(1, 0, 2).reshape(128, KC * N)
    ).astype(bf16)

    in_maps = []
    for c in range(NCORES):
        sl = slice(c * SH, (c + 1) * SH)
        wv = (W_g1[:, sl] * W_g2[sl, 0][None, :]).astype(bf16)  # [D1, SH]
        # swizzle so chunk k sits at cols k*SH..(k+1)*SH of a [128, *] image
        wsw = np.ascontiguousarray(
            wv.reshape(KC, 128, SH).transpose(1, 0, 2).reshape(128, KC * SH)
        )
        in_maps.append({"xb": xb, "wv": wsw})
    return in_maps, A


def epilogue(t_parts, A, inputs):
    f32 = np.float32
    b_g1 = np.asarray(inputs["b_g1"], f32)
    W_g2 = np.asarray(inputs["W_g2"], f32)
    b_g2 = np.asarray(inputs["b_g2"], f32)
    W_head = np.asarray(inputs["W_head"], f32)
    b_head = np.asarray(inputs["b_head"], f32)
    u = np.add.reduce([p.astype(f32) for p in t_parts])  # [14,1] un-aggregated
    t_full = A @ u + np.float32(b_g1 @ W_g2[:, 0])  # conv2 input = x1 @ W_g2
    x2 = A @ t_full + b_g2[0]
    y = float(x2[:, 0] @ W_head[:, 0]) + float(b_head[0])
    return np.array([y], dtype=f32)


def run_on_hw(in_maps, trace=False, tmpdir=None):
    from concourse.bass_utils import run_bass_kernel_spmd

    nc = get_nc()
    return run_bass_kernel_spmd(
        nc,
        in_maps,
        core_ids=list(range(NCORES)),
        trace=trace,
        tmpdir=tmpdir,
    )


def kernel(**inputs):
    in_maps, A = build_host_inputs(inputs)
    res = run_on_hw(in_maps, trace=bool(int(os.environ.get("KERNEL_TRACE", "0"))))
    _CACHE["last_result"] = res
    t_parts = [r["t"] for r in res.results]
    return epilogue(t_parts, A, inputs)
